# revision 15
# baseline (speedup 1.0000x reference)
"""Trainium2 Bass kernel for nn_Coefficients: assemble the sparse circuit
coefficient matrix

    out = [ kcl  = [ M | 0 ]                       (N rows)
            kvl  = [ 0 | I_E | -M^T ]              (E rows)
            elem = diag(z) / diag(y) scatter ]     (E rows)

Sharding: core d owns M row-shard M[d*256:(d+1)*256, :], read ONCE and
used for BOTH nonzero M-derived blocks:
  - kcl:  the shard itself, written back from SBUF
  - mtc:  -shard^T = a 256-column slice of the kvl -M^T block,
          produced by PE transpose-mode -> full-bank PSUM -> negating
          DVE copy -> SBUF chunk -> incremental DMA
  - zyo:  per-element diagonal VALUES (z diag, y diag, I ones; one
          [128,12] f32 write) computed from params/kinds on GpSimd; the
          host scatters them onto the diagonals (pure indexing of
          device-produced content).

Data moves as bf16 (correctness gate is rel_err < 2e-2; bf16
round-to-nearest gives ~3e-3). Per-core SDMA transfer bytes: 2 in +
4.01 out = 6.0 MB vs 13.2 MB for the f32 baseline.

Queue layout (each HWDGE ring sustains ~195 GB/s; 3 queues share the 16
SDMA engines):
  - sync/Q1:    g=0 chunk loads, then g=0 mtc chunk writes
  - scalar/Q10: g=1 chunk loads, then g=1 mtc chunk writes
  - gpsimd/Q0:  all kcl chunk writes + the zyo write (sem-blocked kcl
    dispatches stall only this queue, never the load/mtc rings)

params/kinds ride as 8 extra bf16 COLUMNS of the m tensor (cols
4096:4104, replicated per row-group) so no tiny-descriptor input DMAs
exist; the z/y math reads them as views of the last chunk tile.

mtc device layout [128, 8192]: mtc[p, g*4096 + cb*128 + j] =
-M[128g + j, 128cb + p] (g = row-group, cb = column-block). Host
unscrambles with one reshape/transpose - pure indexing.
"""

import numpy as np

N = 2048
E = 4096
W = 2 * E + N  # 10240
D = 8
NR = N // D  # 256 kcl rows / mt cols per core
EC = E // D  # 512 elem rows per core
EXT = 8  # extra m columns carrying params (4) + kinds (4)

_CACHE: dict = {}


def _build(opts=None):
    import concourse.bacc as bacc
    import concourse.tile as tile
    import concourse.mybir as mybir
    from concourse._compat import get_trn_type

    opts = dict(opts or {})
    ppool_bufs = opts.get("ppool_bufs", 8)
    use_bf16 = opts.get("dtype", "bf16") == "bf16"
    # column-chunk widths per row group; the last chunk carries the EXT
    # params/kinds columns. (Small lead chunks measured WORSE: DMA completion
    # semaphores have ~2-3us fixed latency, so small loads don't unblock the
    # PE proportionally earlier, and the extra small transfers wreck the ramp.)
    WS = list(opts.get("ws", (2048, 2048)))
    assert sum(WS) == E

    f32 = mybir.dt.float32
    mdt = mybir.dt.bfloat16 if use_bf16 else f32

    nc = bacc.Bacc(
        get_trn_type() or "TRN2",
        target_bir_lowering=False,
        debug=False,
        enable_asserts=False,
        num_devices=D,
    )

    m = nc.dram_tensor("m", [NR, E + EXT], mdt, kind="ExternalInput")

    kcl = nc.dram_tensor("kcl", [NR, E], mdt, kind="ExternalOutput")
    mtc = nc.dram_tensor("mtc", [128, 2 * E], mdt, kind="ExternalOutput")
    zyo = nc.dram_tensor("zyo", [128, 12], f32, kind="ExternalOutput")

    AO = mybir.AluOpType
    NCH = len(WS)
    CS = [sum(WS[:i]) for i in range(NCH)]  # chunk column starts
    PSW = 1024 if use_bf16 else 512  # full 2KB-per-partition psum bank

    def psum_groups(w):
        # split a chunk width into PSW-sized groups + one remainder
        offs, o = [], 0
        while o < w:
            g = min(PSW, w - o)
            offs.append((o, g))
            o += g
        return offs

    with tile.TileContext(nc) as tc:
        with (
            tc.tile_pool(name="cpool", bufs=1) as cpool,
            tc.tile_pool(name="ppool", bufs=ppool_bufs, space="PSUM") as ppool,
        ):
            # ---- identity for PE transpose-mode, FIRST on gpsimd (PE dep)
            ident = cpool.tile([128, 128], mdt)
            nc.gpsimd.memset(ident[:], 0.0)
            nc.gpsimd.affine_select(
                out=ident[:],
                in_=ident[:],
                compare_op=AO.not_equal,
                fill=1.0,
                base=0,
                pattern=[[-1, 128]],
                channel_multiplier=1,
            )

            # ---- M row-shard chunk loads on the HWDGE rings (g -> ring).
            # The last chunk is EXT columns wider and carries params/kinds.
            mch = [[None] * NCH for _ in range(2)]
            for ci in range(NCH):
                w = WS[ci] + (EXT if ci == NCH - 1 else 0)
                for g in range(2):
                    t = cpool.tile([128, w], mdt, tag=f"m{g}{ci}")
                    eng = nc.sync if g == 0 else nc.scalar
                    eng.dma_start(
                        out=t[:],
                        in_=m.ap()[g * 128 : (g + 1) * 128, CS[ci] : CS[ci] + w],
                    )
                    mch[g][ci] = t

            # ---- kcl: ONE dependency-free DRAM->DRAM copy on the SWDGE queue
            # (same transfer bytes as an SBUF->DRAM write, but it needs no
            # load sem, so Q0 streams from the start of the window while the
            # rings carry loads; the rings then only carry loads + mtc).
            nc.gpsimd.dma_start(out=kcl.ap()[:, :], in_=m.ap()[:, 0:E])

            # ---- -M^T column slice: PE transposes chunks as they land; DVE
            # (nothing else on it) drains psum banks with negation into
            # staging tiles; every psum group DMAs out immediately, groups
            # alternating across BOTH rings so the rings drain evenly and
            # both finish with a small (0.25 MB) final write.
            mgrp = 0
            for ci in range(NCH):
                for g in range(2):
                    for o, w in psum_groups(WS[ci]):
                        ps = ppool.tile([128, w], mdt)
                        for jj in range(w // 128):
                            lo = o + jj * 128
                            nc.tensor.transpose(
                                out=ps[:, jj * 128 : (jj + 1) * 128],
                                in_=mch[g][ci][:, lo : lo + 128],
                                identity=ident[:],
                            )
                        mt_st = cpool.tile([128, w], mdt, tag=f"t{g}{ci}{o}")
                        nc.vector.tensor_scalar(
                            mt_st[:], ps[:], -1.0, None, op0=AO.mult
                        )
                        eng = nc.sync if mgrp % 2 == 0 else nc.scalar
                        mgrp += 1
                        f0 = g * E + CS[ci] + o
                        eng.dma_start(out=mtc.ap()[:, f0 : f0 + w], in_=mt_st[:])

            # ---- diagonal values on GpSimd from views of the last g0 chunk
            last = mch[0][NCH - 1]
            lw = WS[NCH - 1]
            pv = last[:, lw : lw + 4]  # params
            kv = last[:, lw + 4 : lw + 8]  # kinds

            zy = cpool.tile([128, 12], f32)
            nc.gpsimd.memset(zy[:, 8:12], 1.0)  # I_E diag ones

            rm = cpool.tile([128, 4], f32)
            im = cpool.tile([128, 4], f32)
            vm = cpool.tile([128, 4], f32)
            sm = cpool.tile([128, 4], f32)
            onm = cpool.tile([128, 4], f32)
            offm = cpool.tile([128, 4], f32)
            t0 = cpool.tile([128, 4], f32)
            t1 = cpool.tile([128, 4], f32)
            pf = cpool.tile([128, 4], f32)

            nc.gpsimd.tensor_scalar(pf[:], pv, 1.0, None, op0=AO.mult)  # -> f32
            nc.gpsimd.tensor_scalar(rm[:], kv, 0.0, None, op0=AO.is_equal)
            nc.gpsimd.tensor_scalar(im[:], kv, 1.0, None, op0=AO.is_equal)
            nc.gpsimd.tensor_scalar(vm[:], kv, 2.0, None, op0=AO.is_equal)
            nc.gpsimd.tensor_scalar(sm[:], kv, 3.0, None, op0=AO.is_equal)
            nc.gpsimd.tensor_scalar(onm[:], pf[:], 0.0, None, op0=AO.is_gt)
            nc.gpsimd.tensor_scalar(offm[:], pf[:], 0.0, None, op0=AO.is_le)
            # z = vc + sw*off - r*params
            nc.gpsimd.tensor_tensor(t0[:], sm[:], offm[:], op=AO.mult)
            nc.gpsimd.tensor_tensor(t0[:], vm[:], t0[:], op=AO.add)
            nc.gpsimd.tensor_tensor(t1[:], rm[:], pf[:], op=AO.mult)
            nc.gpsimd.tensor_tensor(zy[:, 0:4], t0[:], t1[:], op=AO.subtract)
            # y = r + ivs + sw*on
            nc.gpsimd.tensor_tensor(t0[:], sm[:], onm[:], op=AO.mult)
            nc.gpsimd.tensor_tensor(t0[:], im[:], t0[:], op=AO.add)
            nc.gpsimd.tensor_tensor(zy[:, 4:8], rm[:], t0[:], op=AO.add)
            nc.gpsimd.dma_start(out=zyo.ap()[:, :], in_=zy[:])

    nc.compile()
    return nc


def _get_nc(opts=None):
    key = ("nc", tuple(sorted((opts or {}).items())))
    if key not in _CACHE:
        _CACHE[key] = _build(opts)
    return _CACHE[key]


def _in_maps(M, params, kinds, use_bf16):
    if use_bf16:
        import ml_dtypes

        dt = ml_dtypes.bfloat16
    else:
        dt = np.float32
    pk = np.empty((128, EXT), dtype=np.float32)
    maps = []
    for d in range(D):
        pk[:, 0:4] = params[d * EC : (d + 1) * EC].reshape(4, 128).T
        pk[:, 4:8] = kinds[d * EC : (d + 1) * EC].reshape(4, 128).T
        m_ext = np.empty((NR, E + EXT), dtype=dt)
        m_ext[:, 0:E] = M[d * NR : (d + 1) * NR, :].astype(dt)
        m_ext[0:128, E:] = pk.astype(dt)
        m_ext[128:256, E:] = pk.astype(dt)
        maps.append({"m": m_ext})
    return maps


def kernel(M, params, kinds, _trace=False, _trace_kwargs=None, _opts=None):
    from concourse.bass_utils import run_bass_kernel_spmd

    M = np.ascontiguousarray(np.asarray(M, dtype=np.float32))
    params = np.ascontiguousarray(np.asarray(params, dtype=np.float32))
    kinds = np.ascontiguousarray(np.asarray(kinds, dtype=np.int32))
    assert M.shape == (N, E) and params.shape == (E,) and kinds.shape == (E,)

    opts = dict(_opts or {})
    use_bf16 = opts.get("dtype", "bf16") == "bf16"
    nc = _get_nc(opts)
    res = run_bass_kernel_spmd(
        nc,
        _in_maps(M, params, kinds, use_bf16),
        core_ids=list(range(D)),
        trace=_trace,
        **(_trace_kwargs or {}),
    )
    out = np.zeros((N + 2 * E, W), np.float32)
    for d in range(D):
        r = res.results[d]
        # kcl block: rows of M
        out[d * NR : (d + 1) * NR, 0:E] = r["kcl"]
        # kvl -M^T block: column slice [E, 256] for this core's nodes.
        # mtc[p, g*4096 + cb*128 + j] = -M[128g+j, 128cb+p]
        v = np.asarray(r["mtc"]).reshape(128, 2, 32, 128)
        mts = v.transpose(2, 0, 1, 3).reshape(E, NR)
        out[N : N + E, 2 * E + d * NR : 2 * E + (d + 1) * NR] = mts
        # diagonals: zyo = [z | y | ones], value layout r = c*128 + p
        gs = d * EC + np.arange(EC)
        zy = r["zyo"]
        z_flat = zy[:, 0:4].T.reshape(EC)
        y_flat = zy[:, 4:8].T.reshape(EC)
        o_flat = zy[:, 8:12].T.reshape(EC)
        out[N + gs, E + gs] = o_flat  # I_E diag in kvl rows
        out[N + E + gs, gs] = z_flat  # elem z diag
        out[N + E + gs, E + gs] = y_flat  # elem y diag
    if _trace:
        _CACHE["last_result"] = res
    return out


# revision 18
# speedup vs baseline: 1.0467x; 1.0467x over previous
"""Trainium2 Bass kernel for nn_Coefficients: assemble the sparse circuit
coefficient matrix

    out = [ kcl  = [ M | 0 ]                       (N rows)
            kvl  = [ 0 | I_E | -M^T ]              (E rows)
            elem = diag(z) / diag(y) scatter ]     (E rows)

Sharding: core d owns M row-shard M[d*256:(d+1)*256, :], read ONCE and
used for BOTH nonzero M-derived blocks:
  - kcl:  the shard itself, written back from SBUF
  - mtc:  -shard^T = a 256-column slice of the kvl -M^T block,
          produced by PE transpose-mode -> full-bank PSUM -> negating
          DVE copy -> SBUF chunk -> incremental DMA
  - zyo:  per-element diagonal VALUES (z diag, y diag, I ones; one
          [128,12] f32 write) computed from params/kinds on GpSimd; the
          host scatters them onto the diagonals (pure indexing of
          device-produced content).

Data moves as bf16 (correctness gate is rel_err < 2e-2; bf16
round-to-nearest gives ~3e-3). Per-core SDMA transfer bytes: 2 in +
4.01 out = 6.0 MB vs 13.2 MB for the f32 baseline.

Queue layout (each HWDGE ring sustains ~195 GB/s; 3 queues share the 16
SDMA engines):
  - sync/Q1:    g=0 chunk loads, then g=0 mtc chunk writes
  - scalar/Q10: g=1 chunk loads, then g=1 mtc chunk writes
  - gpsimd/Q0:  all kcl chunk writes + the zyo write (sem-blocked kcl
    dispatches stall only this queue, never the load/mtc rings)

params/kinds ride as 8 extra bf16 COLUMNS of the m tensor (cols
4096:4104, replicated per row-group) so no tiny-descriptor input DMAs
exist; the z/y math reads them as views of the last chunk tile.

mtc device layout [128, 8192]: mtc[p, g*4096 + cb*128 + j] =
-M[128g + j, 128cb + p] (g = row-group, cb = column-block). Host
unscrambles with one reshape/transpose - pure indexing.
"""

import numpy as np

N = 2048
E = 4096
W = 2 * E + N  # 10240
D = 8
NR = N // D  # 256 kcl rows / mt cols per core
EC = E // D  # 512 elem rows per core
EXT = 8  # extra m columns carrying params (4) + kinds (4)

_CACHE: dict = {}


def _build(opts=None):
    import concourse.bacc as bacc
    import concourse.tile as tile
    import concourse.mybir as mybir
    from concourse._compat import get_trn_type

    opts = dict(opts or {})
    ppool_bufs = opts.get("ppool_bufs", 8)
    use_bf16 = opts.get("dtype", "bf16") == "bf16"
    # column-chunk widths per row group; the last chunk carries the EXT
    # params/kinds columns. (Small lead chunks measured WORSE: DMA completion
    # semaphores have ~2-3us fixed latency, so small loads don't unblock the
    # PE proportionally earlier, and the extra small transfers wreck the ramp.)
    WS = list(opts.get("ws", (2048, 2048)))
    assert sum(WS) == E

    f32 = mybir.dt.float32
    mdt = mybir.dt.bfloat16 if use_bf16 else f32

    nc = bacc.Bacc(
        get_trn_type() or "TRN2",
        target_bir_lowering=False,
        debug=False,
        enable_asserts=False,
        num_devices=D,
    )

    m = nc.dram_tensor("m", [NR, E + EXT], mdt, kind="ExternalInput")

    kcl = nc.dram_tensor("kcl", [NR, E], mdt, kind="ExternalOutput")
    mtc = nc.dram_tensor("mtc", [128, 2 * E], mdt, kind="ExternalOutput")
    zyo = nc.dram_tensor("zyo", [128, 12], f32, kind="ExternalOutput")

    AO = mybir.AluOpType
    NCH = len(WS)
    CS = [sum(WS[:i]) for i in range(NCH)]  # chunk column starts
    PSW = 1024 if use_bf16 else 512  # full 2KB-per-partition psum bank

    def psum_groups(w):
        # split a chunk width into PSW-sized groups + one remainder
        offs, o = [], 0
        while o < w:
            g = min(PSW, w - o)
            offs.append((o, g))
            o += g
        return offs

    with tile.TileContext(nc) as tc:
        with (
            tc.tile_pool(name="cpool", bufs=1) as cpool,
            tc.tile_pool(name="ppool", bufs=ppool_bufs, space="PSUM") as ppool,
        ):
            # ---- identity for PE transpose-mode, FIRST on gpsimd (PE dep)
            ident = cpool.tile([128, 128], mdt)
            nc.gpsimd.memset(ident[:], 0.0)
            nc.gpsimd.affine_select(
                out=ident[:],
                in_=ident[:],
                compare_op=AO.not_equal,
                fill=1.0,
                base=0,
                pattern=[[-1, 128]],
                channel_multiplier=1,
            )

            # ---- M row-shard chunk loads on the HWDGE rings (g -> ring).
            # The last chunk is EXT columns wider and carries params/kinds.
            mch = [[None] * NCH for _ in range(2)]
            for ci in range(NCH):
                w = WS[ci] + (EXT if ci == NCH - 1 else 0)
                for g in range(2):
                    t = cpool.tile([128, w], mdt, tag=f"m{g}{ci}")
                    eng = nc.sync if g == 0 else nc.scalar
                    eng.dma_start(
                        out=t[:],
                        in_=m.ap()[g * 128 : (g + 1) * 128, CS[ci] : CS[ci] + w],
                    )
                    mch[g][ci] = t

            # ---- kcl writes: last chunks on the HWDGE rings (right after the
            # loads), earlier chunks on the gpsimd SWDGE queue. (A
            # dependency-free DRAM->DRAM kcl copy on Q0 measured WORSE: SDMA
            # round-robin is per-packet, so its 8KB descriptors steal ~2/3 of
            # the engine visits from the 4KB-descriptor loads and starve the
            # PE dependency chain.)
            for g in range(2):
                ci = NCH - 1
                eng = nc.sync if g == 0 else nc.scalar
                eng.dma_start(
                    out=kcl.ap()[g * 128 : (g + 1) * 128, CS[ci] : CS[ci] + WS[ci]],
                    in_=mch[g][ci][:, 0 : WS[ci]],
                )
            for ci in range(NCH - 1):
                for g in range(2):
                    nc.gpsimd.dma_start(
                        out=kcl.ap()[g * 128 : (g + 1) * 128, CS[ci] : CS[ci] + WS[ci]],
                        in_=mch[g][ci][:, 0 : WS[ci]],
                    )

            # ---- -M^T column slice: PE transposes chunks as they land; DVE
            # (nothing else on it) drains psum banks with negation into
            # staging tiles; every psum group DMAs out immediately, groups
            # alternating across BOTH rings so the rings drain evenly and
            # both finish with a small (0.25 MB) final write.
            # sync/Q1 reliably starts ~2us before scalar/Q10 (SDMA engines owe
            # Q1's first packet backlog), so Q1 gets 5 of the 8 groups,
            # including the last-ready one.
            MTC_RING = [0, 1, 0, 1, 0, 1, 0, 0]
            mgrp = 0
            for ci in range(NCH):
                for g in range(2):
                    for o, w in psum_groups(WS[ci]):
                        ps = ppool.tile([128, w], mdt)
                        for jj in range(w // 128):
                            lo = o + jj * 128
                            nc.tensor.transpose(
                                out=ps[:, jj * 128 : (jj + 1) * 128],
                                in_=mch[g][ci][:, lo : lo + 128],
                                identity=ident[:],
                            )
                        mt_st = cpool.tile([128, w], mdt, tag=f"t{g}{ci}{o}")
                        nc.vector.tensor_scalar(
                            mt_st[:], ps[:], -1.0, None, op0=AO.mult
                        )
                        eng = nc.sync if MTC_RING[mgrp % 8] == 0 else nc.scalar
                        mgrp += 1
                        f0 = g * E + CS[ci] + o
                        eng.dma_start(out=mtc.ap()[:, f0 : f0 + w], in_=mt_st[:])

            # ---- diagonal values on GpSimd from views of the last g0 chunk
            last = mch[0][NCH - 1]
            lw = WS[NCH - 1]
            pv = last[:, lw : lw + 4]  # params
            kv = last[:, lw + 4 : lw + 8]  # kinds

            zy = cpool.tile([128, 12], f32)
            nc.gpsimd.memset(zy[:, 8:12], 1.0)  # I_E diag ones

            rm = cpool.tile([128, 4], f32)
            im = cpool.tile([128, 4], f32)
            vm = cpool.tile([128, 4], f32)
            sm = cpool.tile([128, 4], f32)
            onm = cpool.tile([128, 4], f32)
            offm = cpool.tile([128, 4], f32)
            t0 = cpool.tile([128, 4], f32)
            t1 = cpool.tile([128, 4], f32)
            pf = cpool.tile([128, 4], f32)

            nc.gpsimd.tensor_scalar(pf[:], pv, 1.0, None, op0=AO.mult)  # -> f32
            nc.gpsimd.tensor_scalar(rm[:], kv, 0.0, None, op0=AO.is_equal)
            nc.gpsimd.tensor_scalar(im[:], kv, 1.0, None, op0=AO.is_equal)
            nc.gpsimd.tensor_scalar(vm[:], kv, 2.0, None, op0=AO.is_equal)
            nc.gpsimd.tensor_scalar(sm[:], kv, 3.0, None, op0=AO.is_equal)
            nc.gpsimd.tensor_scalar(onm[:], pf[:], 0.0, None, op0=AO.is_gt)
            nc.gpsimd.tensor_scalar(offm[:], pf[:], 0.0, None, op0=AO.is_le)
            # z = vc + sw*off - r*params
            nc.gpsimd.tensor_tensor(t0[:], sm[:], offm[:], op=AO.mult)
            nc.gpsimd.tensor_tensor(t0[:], vm[:], t0[:], op=AO.add)
            nc.gpsimd.tensor_tensor(t1[:], rm[:], pf[:], op=AO.mult)
            nc.gpsimd.tensor_tensor(zy[:, 0:4], t0[:], t1[:], op=AO.subtract)
            # y = r + ivs + sw*on
            nc.gpsimd.tensor_tensor(t0[:], sm[:], onm[:], op=AO.mult)
            nc.gpsimd.tensor_tensor(t0[:], im[:], t0[:], op=AO.add)
            nc.gpsimd.tensor_tensor(zy[:, 4:8], rm[:], t0[:], op=AO.add)
            nc.scalar.dma_start(out=zyo.ap()[:, :], in_=zy[:])

    nc.compile()
    return nc


def _get_nc(opts=None):
    key = ("nc", tuple(sorted((opts or {}).items())))
    if key not in _CACHE:
        _CACHE[key] = _build(opts)
    return _CACHE[key]


def _in_maps(M, params, kinds, use_bf16):
    if use_bf16:
        import ml_dtypes

        dt = ml_dtypes.bfloat16
    else:
        dt = np.float32
    pk = np.empty((128, EXT), dtype=np.float32)
    maps = []
    for d in range(D):
        pk[:, 0:4] = params[d * EC : (d + 1) * EC].reshape(4, 128).T
        pk[:, 4:8] = kinds[d * EC : (d + 1) * EC].reshape(4, 128).T
        m_ext = np.empty((NR, E + EXT), dtype=dt)
        m_ext[:, 0:E] = M[d * NR : (d + 1) * NR, :].astype(dt)
        m_ext[0:128, E:] = pk.astype(dt)
        m_ext[128:256, E:] = pk.astype(dt)
        maps.append({"m": m_ext})
    return maps


def kernel(M, params, kinds, _trace=False, _trace_kwargs=None, _opts=None):
    from concourse.bass_utils import run_bass_kernel_spmd

    M = np.ascontiguousarray(np.asarray(M, dtype=np.float32))
    params = np.ascontiguousarray(np.asarray(params, dtype=np.float32))
    kinds = np.ascontiguousarray(np.asarray(kinds, dtype=np.int32))
    assert M.shape == (N, E) and params.shape == (E,) and kinds.shape == (E,)

    opts = dict(_opts or {})
    use_bf16 = opts.get("dtype", "bf16") == "bf16"
    nc = _get_nc(opts)
    res = run_bass_kernel_spmd(
        nc,
        _in_maps(M, params, kinds, use_bf16),
        core_ids=list(range(D)),
        trace=_trace,
        **(_trace_kwargs or {}),
    )
    out = np.zeros((N + 2 * E, W), np.float32)
    for d in range(D):
        r = res.results[d]
        # kcl block: rows of M
        out[d * NR : (d + 1) * NR, 0:E] = r["kcl"]
        # kvl -M^T block: column slice [E, 256] for this core's nodes.
        # mtc[p, g*4096 + cb*128 + j] = -M[128g+j, 128cb+p]
        v = np.asarray(r["mtc"]).reshape(128, 2, 32, 128)
        mts = v.transpose(2, 0, 1, 3).reshape(E, NR)
        out[N : N + E, 2 * E + d * NR : 2 * E + (d + 1) * NR] = mts
        # diagonals: zyo = [z | y | ones], value layout r = c*128 + p
        gs = d * EC + np.arange(EC)
        zy = r["zyo"]
        z_flat = zy[:, 0:4].T.reshape(EC)
        y_flat = zy[:, 4:8].T.reshape(EC)
        o_flat = zy[:, 8:12].T.reshape(EC)
        out[N + gs, E + gs] = o_flat  # I_E diag in kvl rows
        out[N + E + gs, gs] = z_flat  # elem z diag
        out[N + E + gs, E + gs] = y_flat  # elem y diag
    if _trace:
        _CACHE["last_result"] = res
    return out


# revision 20
# speedup vs baseline: 1.1295x; 1.0792x over previous
"""Trainium2 Bass kernel for nn_Coefficients: assemble the sparse circuit
coefficient matrix

    out = [ kcl  = [ M | 0 ]                       (N rows)
            kvl  = [ 0 | I_E | -M^T ]              (E rows)
            elem = diag(z) / diag(y) scatter ]     (E rows)

Sharding: core d owns M row-shard M[d*256:(d+1)*256, :], read ONCE and
used for BOTH nonzero M-derived blocks:
  - kcl:  the shard itself, written back from SBUF
  - mtc:  -shard^T = a 256-column slice of the kvl -M^T block,
          produced by PE transpose-mode -> full-bank PSUM -> negating
          DVE copy -> SBUF chunk -> incremental DMA
  - zyo:  per-element diagonal VALUES (z diag, y diag, I ones; one
          [128,12] f32 write) computed from params/kinds on GpSimd; the
          host scatters them onto the diagonals (pure indexing of
          device-produced content).

Data moves as bf16 (correctness gate is rel_err < 2e-2; bf16
round-to-nearest gives ~3e-3). Per-core SDMA transfer bytes: 2 in +
4.01 out = 6.0 MB vs 13.2 MB for the f32 baseline.

Queue layout (each HWDGE ring sustains ~195 GB/s; 3 queues share the 16
SDMA engines):
  - sync/Q1:    g=0 chunk loads, then g=0 mtc chunk writes
  - scalar/Q10: g=1 chunk loads, then g=1 mtc chunk writes
  - gpsimd/Q0:  all kcl chunk writes + the zyo write (sem-blocked kcl
    dispatches stall only this queue, never the load/mtc rings)

params/kinds ride as 8 extra bf16 COLUMNS of the m tensor (cols
4096:4104, replicated per row-group) so no tiny-descriptor input DMAs
exist; the z/y math reads them as views of the last chunk tile.

mtc device layout [128, 8192]: mtc[p, g*4096 + cb*128 + j] =
-M[128g + j, 128cb + p] (g = row-group, cb = column-block). Host
unscrambles with one reshape/transpose - pure indexing.
"""

import numpy as np

N = 2048
E = 4096
W = 2 * E + N  # 10240
D = 8
NR = N // D  # 256 kcl rows / mt cols per core
EC = E // D  # 512 elem rows per core
EXT = 8  # extra m columns carrying params (4) + kinds (4)

_CACHE: dict = {}


def _build(opts=None):
    import concourse.bacc as bacc
    import concourse.tile as tile
    import concourse.mybir as mybir
    from concourse._compat import get_trn_type

    opts = dict(opts or {})
    ppool_bufs = opts.get("ppool_bufs", 8)
    use_bf16 = opts.get("dtype", "bf16") == "bf16"
    # column-chunk widths per row group; the last chunk carries the EXT
    # params/kinds columns. (Small lead chunks measured WORSE: DMA completion
    # semaphores have ~2-3us fixed latency, so small loads don't unblock the
    # PE proportionally earlier, and the extra small transfers wreck the ramp.)
    WS = list(opts.get("ws", (2048, 2048)))
    assert sum(WS) == E

    f32 = mybir.dt.float32
    mdt = mybir.dt.bfloat16 if use_bf16 else f32

    nc = bacc.Bacc(
        get_trn_type() or "TRN2",
        target_bir_lowering=False,
        debug=False,
        enable_asserts=False,
        num_devices=D,
    )

    m = nc.dram_tensor("m", [NR, E + EXT], mdt, kind="ExternalInput")

    kcl = nc.dram_tensor("kcl", [NR, E], mdt, kind="ExternalOutput")
    mtc = nc.dram_tensor("mtc", [128, 2 * E], mdt, kind="ExternalOutput")
    zyo = nc.dram_tensor("zyo", [128, 12], f32, kind="ExternalOutput")

    AO = mybir.AluOpType
    NCH = len(WS)
    CS = [sum(WS[:i]) for i in range(NCH)]  # chunk column starts
    PSW = 1024 if use_bf16 else 512  # full 2KB-per-partition psum bank

    def psum_groups(w):
        # split a chunk width into PSW-sized groups + one remainder
        offs, o = [], 0
        while o < w:
            g = min(PSW, w - o)
            offs.append((o, g))
            o += g
        return offs

    with tile.TileContext(nc) as tc:
        with (
            tc.tile_pool(name="cpool", bufs=1) as cpool,
            tc.tile_pool(name="ppool", bufs=ppool_bufs, space="PSUM") as ppool,
        ):
            # ---- identity for PE transpose-mode, FIRST on gpsimd (PE dep)
            ident = cpool.tile([128, 128], mdt)
            nc.gpsimd.memset(ident[:], 0.0)
            nc.gpsimd.affine_select(
                out=ident[:],
                in_=ident[:],
                compare_op=AO.not_equal,
                fill=1.0,
                base=0,
                pattern=[[-1, 128]],
                channel_multiplier=1,
            )

            # ---- M row-shard chunk loads on the HWDGE rings (g -> ring).
            # The last chunk is EXT columns wider and carries params/kinds.
            mch = [[None] * NCH for _ in range(2)]
            for ci in range(NCH):
                w = WS[ci] + (EXT if ci == NCH - 1 else 0)
                for g in range(2):
                    t = cpool.tile([128, w], mdt, tag=f"m{g}{ci}")
                    eng = nc.sync if g == 0 else nc.scalar
                    eng.dma_start(
                        out=t[:],
                        in_=m.ap()[g * 128 : (g + 1) * 128, CS[ci] : CS[ci] + w],
                    )
                    mch[g][ci] = t

            # ---- kcl writes: last chunks on the HWDGE rings (right after the
            # loads), earlier chunks on the gpsimd SWDGE queue. (A
            # dependency-free DRAM->DRAM kcl copy on Q0 measured WORSE: SDMA
            # round-robin is per-packet, so its 8KB descriptors steal ~2/3 of
            # the engine visits from the 4KB-descriptor loads and starve the
            # PE dependency chain.)
            for g in range(2):
                ci = NCH - 1
                eng = nc.sync if g == 0 else nc.scalar
                eng.dma_start(
                    out=kcl.ap()[g * 128 : (g + 1) * 128, CS[ci] : CS[ci] + WS[ci]],
                    in_=mch[g][ci][:, 0 : WS[ci]],
                )
            for ci in range(NCH - 1):
                for g in range(2):
                    nc.gpsimd.dma_start(
                        out=kcl.ap()[g * 128 : (g + 1) * 128, CS[ci] : CS[ci] + WS[ci]],
                        in_=mch[g][ci][:, 0 : WS[ci]],
                    )

            # ---- -M^T column slice: PE transposes chunks as they land; DVE
            # (nothing else on it) drains psum banks with negation into
            # staging tiles; every psum group DMAs out immediately, groups
            # alternating across BOTH rings so the rings drain evenly and
            # both finish with a small (0.25 MB) final write.
            # (A 5/3 split compensating Q10's ~2us-late start measured worse
            # than plain alternation; keep the even split.)
            MTC_RING = [0, 1, 0, 1, 0, 1, 0, 1]
            mgrp = 0
            for ci in range(NCH):
                for g in range(2):
                    for o, w in psum_groups(WS[ci]):
                        ps = ppool.tile([128, w], mdt)
                        for jj in range(w // 128):
                            lo = o + jj * 128
                            nc.tensor.transpose(
                                out=ps[:, jj * 128 : (jj + 1) * 128],
                                in_=mch[g][ci][:, lo : lo + 128],
                                identity=ident[:],
                            )
                        mt_st = cpool.tile([128, w], mdt, tag=f"t{g}{ci}{o}")
                        nc.vector.tensor_scalar(
                            mt_st[:], ps[:], -1.0, None, op0=AO.mult
                        )
                        eng = nc.sync if MTC_RING[mgrp % 8] == 0 else nc.scalar
                        mgrp += 1
                        f0 = g * E + CS[ci] + o
                        eng.dma_start(out=mtc.ap()[:, f0 : f0 + w], in_=mt_st[:])

            # ---- diagonal values on GpSimd from views of the last g0 chunk
            last = mch[0][NCH - 1]
            lw = WS[NCH - 1]
            pv = last[:, lw : lw + 4]  # params
            kv = last[:, lw + 4 : lw + 8]  # kinds

            zy = cpool.tile([128, 12], f32)
            nc.gpsimd.memset(zy[:, 8:12], 1.0)  # I_E diag ones

            rm = cpool.tile([128, 4], f32)
            im = cpool.tile([128, 4], f32)
            vm = cpool.tile([128, 4], f32)
            sm = cpool.tile([128, 4], f32)
            onm = cpool.tile([128, 4], f32)
            offm = cpool.tile([128, 4], f32)
            t0 = cpool.tile([128, 4], f32)
            t1 = cpool.tile([128, 4], f32)
            pf = cpool.tile([128, 4], f32)

            nc.gpsimd.tensor_scalar(pf[:], pv, 1.0, None, op0=AO.mult)  # -> f32
            nc.gpsimd.tensor_scalar(rm[:], kv, 0.0, None, op0=AO.is_equal)
            nc.gpsimd.tensor_scalar(im[:], kv, 1.0, None, op0=AO.is_equal)
            nc.gpsimd.tensor_scalar(vm[:], kv, 2.0, None, op0=AO.is_equal)
            nc.gpsimd.tensor_scalar(sm[:], kv, 3.0, None, op0=AO.is_equal)
            nc.gpsimd.tensor_scalar(onm[:], pf[:], 0.0, None, op0=AO.is_gt)
            nc.gpsimd.tensor_scalar(offm[:], pf[:], 0.0, None, op0=AO.is_le)
            # z = vc + sw*off - r*params
            nc.gpsimd.tensor_tensor(t0[:], sm[:], offm[:], op=AO.mult)
            nc.gpsimd.tensor_tensor(t0[:], vm[:], t0[:], op=AO.add)
            nc.gpsimd.tensor_tensor(t1[:], rm[:], pf[:], op=AO.mult)
            nc.gpsimd.tensor_tensor(zy[:, 0:4], t0[:], t1[:], op=AO.subtract)
            # y = r + ivs + sw*on
            nc.gpsimd.tensor_tensor(t0[:], sm[:], onm[:], op=AO.mult)
            nc.gpsimd.tensor_tensor(t0[:], im[:], t0[:], op=AO.add)
            nc.gpsimd.tensor_tensor(zy[:, 4:8], rm[:], t0[:], op=AO.add)
            nc.gpsimd.dma_start(out=zyo.ap()[:, :], in_=zy[:])

    nc.compile()
    return nc


def _get_nc(opts=None):
    key = ("nc", tuple(sorted((opts or {}).items())))
    if key not in _CACHE:
        _CACHE[key] = _build(opts)
    return _CACHE[key]


def _in_maps(M, params, kinds, use_bf16):
    if use_bf16:
        import ml_dtypes

        dt = ml_dtypes.bfloat16
    else:
        dt = np.float32
    pk = np.empty((128, EXT), dtype=np.float32)
    maps = []
    for d in range(D):
        pk[:, 0:4] = params[d * EC : (d + 1) * EC].reshape(4, 128).T
        pk[:, 4:8] = kinds[d * EC : (d + 1) * EC].reshape(4, 128).T
        m_ext = np.empty((NR, E + EXT), dtype=dt)
        m_ext[:, 0:E] = M[d * NR : (d + 1) * NR, :].astype(dt)
        m_ext[0:128, E:] = pk.astype(dt)
        m_ext[128:256, E:] = pk.astype(dt)
        maps.append({"m": m_ext})
    return maps


def kernel(M, params, kinds, _trace=False, _trace_kwargs=None, _opts=None):
    from concourse.bass_utils import run_bass_kernel_spmd

    M = np.ascontiguousarray(np.asarray(M, dtype=np.float32))
    params = np.ascontiguousarray(np.asarray(params, dtype=np.float32))
    kinds = np.ascontiguousarray(np.asarray(kinds, dtype=np.int32))
    assert M.shape == (N, E) and params.shape == (E,) and kinds.shape == (E,)

    opts = dict(_opts or {})
    use_bf16 = opts.get("dtype", "bf16") == "bf16"
    nc = _get_nc(opts)
    res = run_bass_kernel_spmd(
        nc,
        _in_maps(M, params, kinds, use_bf16),
        core_ids=list(range(D)),
        trace=_trace,
        **(_trace_kwargs or {}),
    )
    out = np.zeros((N + 2 * E, W), np.float32)
    for d in range(D):
        r = res.results[d]
        # kcl block: rows of M
        out[d * NR : (d + 1) * NR, 0:E] = r["kcl"]
        # kvl -M^T block: column slice [E, 256] for this core's nodes.
        # mtc[p, g*4096 + cb*128 + j] = -M[128g+j, 128cb+p]
        v = np.asarray(r["mtc"]).reshape(128, 2, 32, 128)
        mts = v.transpose(2, 0, 1, 3).reshape(E, NR)
        out[N : N + E, 2 * E + d * NR : 2 * E + (d + 1) * NR] = mts
        # diagonals: zyo = [z | y | ones], value layout r = c*128 + p
        gs = d * EC + np.arange(EC)
        zy = r["zyo"]
        z_flat = zy[:, 0:4].T.reshape(EC)
        y_flat = zy[:, 4:8].T.reshape(EC)
        o_flat = zy[:, 8:12].T.reshape(EC)
        out[N + gs, E + gs] = o_flat  # I_E diag in kvl rows
        out[N + E + gs, gs] = z_flat  # elem z diag
        out[N + E + gs, E + gs] = y_flat  # elem y diag
    if _trace:
        _CACHE["last_result"] = res
    return out
